# revision 36
# baseline (speedup 1.0000x reference)
"""Trainium2 Bass kernel for an AttentionBlock (GroupNorm -> QKV 1x1conv ->
8-head attention over 1024 spatial positions -> proj 1x1conv -> residual).

Input x: (8, 512, 32, 32) f32.  Data-parallel over batch: one batch element
per NeuronCore (8 cores).

Per-core design ([C,N] layout, C on partitions, N=H*W=1024):
  - GroupNorm stats via bn_stats per channel + tiny PE matmuls (selection
    matrices) to combine/broadcast across the 16-channel groups.
  - q,k projections in [o,n] layout; v projection directly transposed [n,o]
    (swap matmul operands); all matmuls bf16 with f32 PSUM accumulation.
  - Attention per head-pair (heads 2t/2t+1 sit at partitions 0-63/64-127 of
    channel tile t): K=64 qk matmuls row-tiled at base partitions 0/64 (2
    concurrently in the PE array), logits in [m,n] layout, exp on ScalarE
    (PSUM->SBUF bf16, 1/8 scale fused), attn@v with a ones column appended
    to v^T so the softmax denominator falls out of the same matmul (row 64).
  - 1/denom via reciprocal_approx_fast; broadcast across partitions with
    GPSIMD partition_broadcast; v-bias and proj-bias folded on the host into
    bp_eff = wp@bv + bp.
  - Software pipelining: qk/exp of head-pair s+1 is issued before attn@v of
    head-pair s so ScalarE (the bottleneck) never starves.
"""

import sys

sys.path.insert(0, "/opt/trn_rl_repo")

import numpy as np
import ml_dtypes

import concourse.bacc as bacc
import concourse.tile as tile
from concourse import mybir
from concourse.bass_utils import run_bass_kernel_spmd

BF16 = ml_dtypes.bfloat16
F32 = mybir.dt.float32
BF = mybir.dt.bfloat16

B, C, HH, WW = 8, 512, 32, 32
N = HH * WW          # 1024
HEADS, HD = 8, 64
G, GS = 32, 16       # groups, channels per group
EPS = 1e-5
NT = C // 128        # 4 channel tiles
MT = N // 128        # 8 position tiles
NC2 = N // 512       # 2 free-dim chunks
ATT_SCALE = 0.125


def _build_nc(repeat=1):
    nc = bacc.Bacc("TRN2", target_bir_lowering=False, debug=False)

    x_d = nc.dram_tensor("x", [C, N], F32, kind="ExternalInput")
    wq_d = nc.dram_tensor("wq_t", [C, C], BF, kind="ExternalInput")
    wk_d = nc.dram_tensor("wk_t", [C, C], BF, kind="ExternalInput")
    wv_d = nc.dram_tensor("wv_t", [C, C], BF, kind="ExternalInput")
    wp_d = nc.dram_tensor("wp_t", [C, C], BF, kind="ExternalInput")
    cst_d = nc.dram_tensor("consts", [128, 28], F32, kind="ExternalInput")
    selT_d = nc.dram_tensor("selT", [8, 128], F32, kind="ExternalInput")
    y_d = nc.dram_tensor("y", [C, N], F32, kind="ExternalOutput")

    with tile.TileContext(nc) as tc:
        with (
            tc.tile_pool(name="sb", bufs=1) as sb,
            tc.tile_pool(name="wk", bufs=2) as wkp,
            tc.tile_pool(name="ps_pr", bufs=4, space="PSUM") as ps_pr,
            tc.tile_pool(name="ps_qk", bufs=2, space="PSUM") as ps_qk,
        ):
          for _rep in range(repeat):
            # ---- engine warmups (no data deps; scheduled early) ----
            warm_w = sb.tile([128, 128], BF, tag="warm_w")
            nc.vector.memset(warm_w, 0.0)
            warm_ps = ps_pr.tile([128, 128], F32, tag="proj", name="warm_ps")
            for _ in range(12):
                nc.tensor.matmul(warm_ps, warm_w, warm_w, start=True, stop=True)
            junk = sb.tile([1, 8], F32, tag="junk")
            nc.vector.memset(junk, 0.0)
            junk2 = sb.tile([1, 8], BF, tag="junk2")
            nc.scalar.activation(out=junk2, in_=junk,
                                 func=mybir.ActivationFunctionType.Exp)
            junk3 = sb.tile([1, 8], F32, tag="junk3")
            nc.scalar.activation(out=junk3, in_=junk,
                                 func=mybir.ActivationFunctionType.Sqrt)

            # ---- input DMAs: x first (longest dependency chain), then the
            # packed constants (one DMA), then weights ----
            x_sb = sb.tile([128, NT, N], F32, tag="x")
            x_r = x_d.ap().rearrange("(t p) n -> p t n", p=128)
            for t in range(NT):
                nc.sync.dma_start(out=x_sb[:, t, :], in_=x_r[:, t, :])

            # staggered PE warmups: keep the HAM clock-gate open until the
            # projections start (no-dep batch ran at t~0; these wait on x t0)
            warm_ps2 = ps_pr.tile([128, 128], F32, tag="proj", name="warm_ps2")
            for _ in range(14):
                nc.tensor.matmul(warm_ps2, x_sb[:, 0, 0:128],
                                 x_sb[:, 0, 128:256], start=True, stop=True)

            cst_sb = sb.tile([128, 28], F32, tag="cst")
            nc.sync.dma_start(out=cst_sb, in_=cst_d.ap())
            selT_sb = sb.tile([8, 128], F32, tag="selT")
            nc.sync.dma_start(out=selT_sb, in_=selT_d.ap())
            sel_sb = cst_sb[:, 0:8]
            gsc_sb = cst_sb[:, 8:12]
            gbi_sb = cst_sb[:, 12:16]
            bq_sb = cst_sb[:, 16:20]
            bk_sb = cst_sb[:, 20:24]
            bp_sb = cst_sb[:, 24:28]

            w_sbs = {}
            for nm, d in (("k", wk_d), ("q", wq_d), ("v", wv_d), ("p", wp_d)):
                w_sb = sb.tile([128, NT, C], BF, tag=f"w{nm}", name=f"w{nm}_sb")
                nc.sync.dma_start(out=w_sb, in_=d.ap().rearrange("(t p) o -> p t o", p=128))
                w_sbs[nm] = w_sb

            # ---- GroupNorm, fully per channel-tile (the 16-channel groups
            # never span a 128-channel tile) ----
            eps_sb = sb.tile([8, 1], F32, tag="eps")
            nc.vector.memset(eps_sb, EPS)
            AB = sb.tile([128, NT, 2], F32, tag="AB")
            xn_bf = sb.tile([128, NT, N], BF, tag="xn")
            for t in range(NT):
                stats_t = wkp.tile([128, 2], F32, tag="stats_t", bufs=2, name="stats_t")
                bst = wkp.tile([128, 2, 6], F32, tag="bst")
                for s in range(2):
                    nc.vector.bn_stats(out=bst[:, s, :], in_=x_sb[:, t, s * 512:(s + 1) * 512])
                mv = wkp.tile([128, 2], F32, tag="mv")
                nc.vector.bn_aggr(out=mv, in_=bst)
                # stats_t = [mean, E2]
                nc.vector.tensor_copy(out=stats_t[:, 0:1], in_=mv[:, 0:1])
                nc.vector.tensor_tensor(out=stats_t[:, 1:2], in0=mv[:, 0:1],
                                        in1=mv[:, 0:1], op=mybir.AluOpType.mult)
                nc.vector.tensor_tensor(out=stats_t[:, 1:2], in0=stats_t[:, 1:2],
                                        in1=mv[:, 1:2], op=mybir.AluOpType.add)
                # group [mean, E2] for this tile's 8 groups
                g_ps = ps_pr.tile([8, 2], F32, tag="proj", name="g_ps")
                nc.tensor.matmul(g_ps, sel_sb, stats_t, start=True, stop=True)
                g_sb = wkp.tile([8, 2], F32, tag="g_sb", bufs=2, name="g_sb")
                nc.vector.tensor_copy(out=g_sb, in_=g_ps)
                gvar = wkp.tile([8, 1], F32, tag="gvar", bufs=2, name="gvar")
                nc.vector.tensor_tensor(out=gvar, in0=g_sb[:, 0:1], in1=g_sb[:, 0:1],
                                        op=mybir.AluOpType.mult)
                nc.vector.tensor_tensor(out=gvar, in0=g_sb[:, 1:2], in1=gvar,
                                        op=mybir.AluOpType.subtract)
                nc.scalar.activation(out=gvar, in_=gvar,
                                     func=mybir.ActivationFunctionType.Sqrt,
                                     bias=eps_sb, scale=1.0)
                nc.vector.reciprocal(out=g_sb[:, 1:2], in_=gvar)
                # broadcast [mean, rstd] back to the tile's 128 channels
                bc_ps = ps_pr.tile([128, 2], F32, tag="proj", name="bc_ps")
                nc.tensor.matmul(bc_ps, selT_sb, g_sb, start=True, stop=True)
                # A = rstd * gn_scale ; B = gn_bias - mean * A
                nc.vector.tensor_tensor(out=AB[:, t, 0:1], in0=bc_ps[:, 1:2],
                                        in1=gsc_sb[:, t:t + 1], op=mybir.AluOpType.mult)
                nc.vector.tensor_tensor(out=AB[:, t, 1:2], in0=bc_ps[:, 0:1],
                                        in1=AB[:, t, 0:1], op=mybir.AluOpType.mult)
                nc.vector.tensor_tensor(out=AB[:, t, 1:2], in0=gbi_sb[:, t:t + 1],
                                        in1=AB[:, t, 1:2], op=mybir.AluOpType.subtract)
                for nc5 in range(NC2):
                    sl = slice(nc5 * 512, (nc5 + 1) * 512)
                    eng = nc.gpsimd if t < 2 else nc.vector
                    eng.tensor_scalar(
                        out=xn_bf[:, t, sl], in0=x_sb[:, t, sl],
                        scalar1=AB[:, t, 0:1], scalar2=AB[:, t, 1:2],
                        op0=mybir.AluOpType.mult, op1=mybir.AluOpType.add)

            warm_ps3 = ps_pr.tile([128, 128], F32, tag="proj", name="warm_ps3")
            for _ in range(6):
                nc.tensor.matmul(warm_ps3, xn_bf[:, 0, 0:128],
                                 xn_bf[:, 0, 128:256], start=True, stop=True)

            # ---- projections (helpers) ----
            q_sb = sb.tile([128, NT, N], BF, tag="q")
            k_sb = sb.tile([128, NT, N], BF, tag="k")

            def emit_qk_proj(dst, wname, bias, ot, on_act=False):
                w_sb = w_sbs[wname]
                for nc5 in range(NC2):
                    pp = ps_pr.tile([128, 512], F32, tag="proj", name=f"pr_{wname}")
                    for kc in range(NT):
                        nc.tensor.matmul(
                            pp, w_sb[:, kc, ot * 128:(ot + 1) * 128],
                            xn_bf[:, kc, nc5 * 512:(nc5 + 1) * 512],
                            start=(kc == 0), stop=(kc == NT - 1))
                    out_sl = dst[:, ot, nc5 * 512:(nc5 + 1) * 512]
                    if on_act:
                        nc.scalar.add(out_sl, pp, bias[:, ot:ot + 1])
                    else:
                        nc.vector.tensor_scalar_add(out_sl, pp, bias[:, ot:ot + 1])

            # v^T projection with ones column per head
            vt_sb = sb.tile([128, MT, HEADS, HD + 1], BF, tag="vt")
            nc.vector.memset(vt_sb[:, :, :, HD:HD + 1], 1.0)

            def emit_vt_proj(mt):
                wv_sb = w_sbs["v"]
                pv = ps_pr.tile([128, 512], F32, tag="proj", name="pr_v")
                for kc in range(NT):
                    nc.tensor.matmul(
                        pv, xn_bf[:, kc, mt * 128:(mt + 1) * 128],
                        wv_sb[:, kc, :], start=(kc == 0), stop=(kc == NT - 1))
                nc.vector.tensor_copy(
                    out=vt_sb[:, mt, :, 0:HD],
                    in_=pv.rearrange("p (h c) -> p h c", h=HEADS))

            # ---- attention blocks ----
            xatt_sb = sb.tile([128, NT, N], BF, tag="xatt")
            # denominator staging: both heads at partition 0 (hardware
            # partition_broadcast only reads absolute partition 0), side by
            # side in the free dim
            rs_sb = sb.tile([32, 2, N], F32, tag="rs")
            nc.vector.memset(rs_sb, 1.0)
            rc_sb = sb.tile([32, 2, N], F32, tag="rc")

            pts_all = {}

            def emit_qk_exp(hp, serial_heads=False):
                hA, hB = 2 * hp, 2 * hp + 1
                if serial_heads:
                    order = [(hA, 0, mt) for mt in range(MT)] + \
                            [(hB, 64, mt) for mt in range(MT)]
                else:
                    order = [(h, lo, mt) for mt in range(MT)
                             for h, lo in ((hA, 0), (hB, 64))]
                for h, lo, mt in order:
                    if True:
                        qk = ps_qk.tile([128, N], F32, tag="qk", name="qk_ps")
                        for nc5 in range(NC2):
                            nc.tensor.matmul(
                                qk[:, nc5 * 512:(nc5 + 1) * 512],
                                k_sb[lo:lo + 64, hp, mt * 128:(mt + 1) * 128],
                                q_sb[lo:lo + 64, hp, nc5 * 512:(nc5 + 1) * 512],
                                start=True, stop=True)
                        pt = wkp.tile([128, N], BF, tag="pt", bufs=30, name="pt")
                        nc.scalar.activation(out=pt, in_=qk,
                                             func=mybir.ActivationFunctionType.Exp,
                                             scale=ATT_SCALE)
                        pts_all[(h, mt)] = pt

            def emit_av(hp, head_major=False):
                hA, hB = 2 * hp, 2 * hp + 1
                if head_major:
                    groups = [[(h, nc5) for nc5 in range(NC2)] for h in (hA, hB)]
                else:
                    groups = [[(h, nc5) for h in (hA, hB)] for nc5 in range(NC2)]
                for grp in groups:
                    avs = {}
                    for h, nc5 in grp:
                        sl = slice(nc5 * 512, (nc5 + 1) * 512)
                        av = ps_pr.tile([HD + 1, 512], F32, tag="proj", name="av_ps")
                        for mt in range(MT):
                            nc.tensor.matmul(
                                av, vt_sb[:, mt, h, :],
                                pts_all[(h, mt)][:, sl],
                                start=(mt == 0), stop=(mt == MT - 1))
                        # denominator row (psum row 64) -> partition 0 slot.
                        # staging slot/cols indexed by the group's varying
                        # coordinate (head for chunk-major, chunk for
                        # head-major)
                        if head_major:
                            slot, scol = nc5, slice(0, 512)
                        else:
                            slot, scol = h % 2, sl
                        nc.vector.tensor_copy(out=rs_sb[0:1, slot, scol],
                                              in_=av[HD:HD + 1, :])
                        avs[(h, nc5)] = av

                    # 1/denom for the group's two halves in one op, then
                    # broadcast across 64 partitions on GPSIMD
                    if head_major:
                        nc.vector.reciprocal_approx_fast(
                            out=rc_sb[0:1, 0:2, 0:512],
                            in_=rs_sb[0:1, 0:2, 0:512])
                    else:
                        gsl = slice(grp[0][1] * 512, (grp[0][1] + 1) * 512)
                        nc.vector.reciprocal_approx_fast(out=rc_sb[0:1, :, gsl],
                                                         in_=rs_sb[0:1, :, gsl])
                    for h, nc5 in grp:
                        sl = slice(nc5 * 512, (nc5 + 1) * 512)
                        lo = 64 * (h % 2)
                        if head_major:
                            slot, scol = nc5, slice(0, 512)
                        else:
                            slot, scol = h % 2, sl
                        rb = wkp.tile([64, 512], F32, tag="rb", bufs=4, name="rb")
                        nc.gpsimd.partition_broadcast(
                            out_ap=rb, in_ap=rc_sb[0:1, slot, scol])
                        nc.vector.tensor_tensor(
                            out=xatt_sb[lo:lo + 64, hp, sl],
                            in0=avs[(h, nc5)][0:HD, :], in1=rb,
                            op=mybir.AluOpType.mult)

            # ---- emission order: software pipeline ----
            emit_qk_proj(k_sb, "k", bk_sb, 0, on_act=True)
            emit_qk_proj(q_sb, "q", bq_sb, 0, on_act=True)
            emit_qk_exp(0)
            for mt in range(0, 4):
                emit_vt_proj(mt)
            emit_qk_proj(k_sb, "k", bk_sb, 1)
            emit_qk_proj(q_sb, "q", bq_sb, 1)
            emit_qk_exp(1)
            for mt in range(4, MT):
                emit_vt_proj(mt)
            emit_av(0)
            emit_qk_proj(k_sb, "k", bk_sb, 2)
            emit_qk_proj(q_sb, "q", bq_sb, 2)
            emit_qk_exp(2)
            emit_av(1)
            emit_qk_proj(k_sb, "k", bk_sb, 3)
            emit_qk_proj(q_sb, "q", bq_sb, 3)
            emit_qk_exp(3)
            emit_av(2)

            # final projection pass 1: accumulate kc=0..2; emitted after the
            # last attn@v so its matmuls fill PE stalls during that block's
            # denominator chains; kc=3 joins in pass 2.
            wp_sb = w_sbs["p"]
            y_part = sb.tile([128, NT, N], F32, tag="y_part")
            for ot in range(NT):
                for nc5 in range(NC2):
                    sl = slice(nc5 * 512, (nc5 + 1) * 512)
                    pq = ps_pr.tile([128, 512], F32, tag="proj", name="pr_p1")
                    for kc in range(NT - 1):
                        nc.tensor.matmul(
                            pq, wp_sb[:, kc, ot * 128:(ot + 1) * 128],
                            xatt_sb[:, kc, sl], start=(kc == 0), stop=(kc == NT - 2))
                    # y_part = pq + bias + x  (bias per-partition, residual)
                    nc.vector.scalar_tensor_tensor(
                        out=y_part[:, ot, sl], in0=pq, scalar=bp_sb[:, ot:ot + 1],
                        in1=x_sb[:, ot, sl], op0=mybir.AluOpType.add,
                        op1=mybir.AluOpType.add)

            emit_av(HEADS // 2 - 1)

            # ---- final projection pass 2 (kc=3) + residual ----
            y_rr = y_d.ap().rearrange("(t p) n -> p t n", p=128)
            for ot in range(NT):
                for nc5 in range(NC2):
                    sl = slice(nc5 * 512, (nc5 + 1) * 512)
                    po = ps_pr.tile([128, 512], F32, tag="proj", name="pr_o")
                    nc.tensor.matmul(
                        po, wp_sb[:, NT - 1, ot * 128:(ot + 1) * 128],
                        xatt_sb[:, NT - 1, sl], start=True, stop=True)
                    yw = wkp.tile([128, 512], F32, tag="yw", bufs=3, name="yw")
                    nc.vector.tensor_tensor(out=yw, in0=po, in1=y_part[:, ot, sl],
                                            op=mybir.AluOpType.add)
                    nc.sync.dma_start(out=y_rr[:, ot, sl], in_=yw)

    nc.compile()
    return nc


_CACHE = {}


def _get_nc(repeat=1):
    key = ("nc", repeat)
    if key not in _CACHE:
        _CACHE[key] = _build_nc(repeat)
    return _CACHE[key]


def _prep_shared(gn_scale, gn_bias, wq, bq, wk, bk, wv, bv, wp, bp):
    def t128(v):
        return np.ascontiguousarray(np.asarray(v, np.float32).reshape(NT, 128).T)

    sel = (np.arange(128)[:, None] // GS == np.arange(8)[None, :]) \
        .astype(np.float32) / GS
    consts = np.concatenate([
        sel, t128(gn_scale), t128(gn_bias), t128(bq), t128(bk),
        t128(np.asarray(wp, np.float64) @ np.asarray(bv, np.float64)
             + np.asarray(bp, np.float64)),
    ], axis=1)
    shared = {
        "wq_t": np.ascontiguousarray(np.asarray(wq, np.float32).T).astype(BF16),
        "wk_t": np.ascontiguousarray(np.asarray(wk, np.float32).T).astype(BF16),
        "wv_t": np.ascontiguousarray(np.asarray(wv, np.float32).T).astype(BF16),
        "wp_t": np.ascontiguousarray(np.asarray(wp, np.float32).T).astype(BF16),
        "consts": np.ascontiguousarray(consts),
        "selT": np.ascontiguousarray(
            (np.arange(8)[:, None] == np.arange(128)[None, :] // GS)
            .astype(np.float32)),
    }
    return shared


def run(inputs, trace=False):
    nc = _get_nc()
    x = np.asarray(inputs["x"], np.float32).reshape(B, C, N)
    shared = _prep_shared(
        inputs["gn_scale"], inputs["gn_bias"], inputs["wq"], inputs["bq"],
        inputs["wk"], inputs["bk"], inputs["wv"], inputs["bv"],
        inputs["wp"], inputs["bp"])
    in_maps = [dict(shared, x=np.ascontiguousarray(x[c])) for c in range(B)]
    try:
        res = run_bass_kernel_spmd(nc, in_maps, core_ids=list(range(B)), trace=trace)
        y = np.stack([res.results[c]["y"] for c in range(B)], axis=0)
    except Exception:
        # transient NRT_EXEC_UNIT_UNRECOVERABLE crashes have been observed on
        # this fabric; a single retry has always succeeded
        res = run_bass_kernel_spmd(nc, in_maps, core_ids=list(range(B)), trace=trace)
        y = np.stack([res.results[c]["y"] for c in range(B)], axis=0)
    return y.reshape(B, C, HH, WW).astype(np.float32), res


def kernel(**inputs) -> np.ndarray:
    y, _ = run(inputs, trace=False)
    return y


# revision 44
# speedup vs baseline: 1.0386x; 1.0386x over previous
"""Trainium2 Bass kernel for an AttentionBlock (GroupNorm -> QKV 1x1conv ->
8-head attention over 1024 spatial positions -> proj 1x1conv -> residual).

Input x: (8, 512, 32, 32) f32.  Data-parallel over batch: one batch element
per NeuronCore (8 cores).

Per-core design ([C,N] layout, C on partitions, N=H*W=1024):
  - GroupNorm stats via bn_stats per channel + tiny PE matmuls (selection
    matrices) to combine/broadcast across the 16-channel groups.
  - q,k projections in [o,n] layout; v projection directly transposed [n,o]
    (swap matmul operands); all matmuls bf16 with f32 PSUM accumulation.
  - Attention per head-pair (heads 2t/2t+1 sit at partitions 0-63/64-127 of
    channel tile t): K=64 qk matmuls row-tiled at base partitions 0/64 (2
    concurrently in the PE array), logits in [m,n] layout, exp on ScalarE
    (PSUM->SBUF bf16, 1/8 scale fused), attn@v with a ones column appended
    to v^T so the softmax denominator falls out of the same matmul (row 64).
  - 1/denom via reciprocal_approx_fast; broadcast across partitions with
    GPSIMD partition_broadcast; v-bias and proj-bias folded on the host into
    bp_eff = wp@bv + bp.
  - Software pipelining: qk/exp of head-pair s+1 is issued before attn@v of
    head-pair s so ScalarE (the bottleneck) never starves.
"""

import sys

sys.path.insert(0, "/opt/trn_rl_repo")

import numpy as np
import ml_dtypes

import concourse.bacc as bacc
import concourse.tile as tile
from concourse import mybir
from concourse.bass_utils import run_bass_kernel_spmd

BF16 = ml_dtypes.bfloat16
F32 = mybir.dt.float32
BF = mybir.dt.bfloat16

B, C, HH, WW = 8, 512, 32, 32
N = HH * WW          # 1024
HEADS, HD = 8, 64
G, GS = 32, 16       # groups, channels per group
EPS = 1e-5
NT = C // 128        # 4 channel tiles
MT = N // 128        # 8 position tiles
NC2 = N // 512       # 2 free-dim chunks
ATT_SCALE = 0.125


def _build_nc(repeat=1):
    nc = bacc.Bacc("TRN2", target_bir_lowering=False, debug=False)

    x_d = nc.dram_tensor("x", [C, N], F32, kind="ExternalInput")
    wq_d = nc.dram_tensor("wq_t", [C, C], BF, kind="ExternalInput")
    wk_d = nc.dram_tensor("wk_t", [C, C], BF, kind="ExternalInput")
    wv_d = nc.dram_tensor("wv_t", [C, C], BF, kind="ExternalInput")
    wp_d = nc.dram_tensor("wp_t", [C, C], BF, kind="ExternalInput")
    cst_d = nc.dram_tensor("consts", [128, 28], F32, kind="ExternalInput")
    selT_d = nc.dram_tensor("selT", [8, 128], F32, kind="ExternalInput")
    y_d = nc.dram_tensor("y", [C, N], F32, kind="ExternalOutput")

    with tile.TileContext(nc) as tc:
        with (
            tc.tile_pool(name="sb", bufs=1) as sb,
            tc.tile_pool(name="wk", bufs=2) as wkp,
            tc.tile_pool(name="ps_pr", bufs=4, space="PSUM") as ps_pr,
            tc.tile_pool(name="ps_qk", bufs=2, space="PSUM") as ps_qk,
        ):
          for _rep in range(repeat):
            # ---- engine warmups (no data deps; scheduled early) ----
            warm_w = sb.tile([128, 128], BF, tag="warm_w")
            nc.vector.memset(warm_w, 0.0)
            warm_ps = ps_pr.tile([128, 128], F32, tag="proj", name="warm_ps")
            for _ in range(12):
                nc.tensor.matmul(warm_ps, warm_w, warm_w, start=True, stop=True)
            junk = sb.tile([1, 8], F32, tag="junk")
            nc.vector.memset(junk, 0.0)
            junk2 = sb.tile([1, 8], BF, tag="junk2")
            nc.scalar.activation(out=junk2, in_=junk,
                                 func=mybir.ActivationFunctionType.Exp)
            junk3 = sb.tile([1, 8], F32, tag="junk3")
            nc.scalar.activation(out=junk3, in_=junk,
                                 func=mybir.ActivationFunctionType.Sqrt)

            # ---- input DMAs: x first (longest dependency chain), then the
            # packed constants (one DMA), then weights ----
            x_sb = sb.tile([128, NT, N], F32, tag="x")
            x_r = x_d.ap().rearrange("(t p) n -> p t n", p=128)
            for t in range(NT):
                nc.sync.dma_start(out=x_sb[:, t, :], in_=x_r[:, t, :])

            # staggered PE warmups: keep the HAM clock-gate open until the
            # projections start (no-dep batch ran at t~0; these wait on x t0)
            warm_ps2 = ps_pr.tile([128, 128], F32, tag="proj", name="warm_ps2")
            for _ in range(14):
                nc.tensor.matmul(warm_ps2, x_sb[:, 0, 0:128],
                                 x_sb[:, 0, 128:256], start=True, stop=True)

            cst_sb = sb.tile([128, 28], F32, tag="cst")
            nc.sync.dma_start(out=cst_sb, in_=cst_d.ap())
            selT_sb = sb.tile([8, 128], F32, tag="selT")
            nc.sync.dma_start(out=selT_sb, in_=selT_d.ap())
            sel_sb = cst_sb[:, 0:8]
            gsc_sb = cst_sb[:, 8:12]
            gbi_sb = cst_sb[:, 12:16]
            bq_sb = cst_sb[:, 16:20]
            bk_sb = cst_sb[:, 20:24]
            bp_sb = cst_sb[:, 24:28]

            w_sbs = {}
            for nm, d in (("k", wk_d), ("q", wq_d), ("v", wv_d), ("p", wp_d)):
                w_sb = sb.tile([128, NT, C], BF, tag=f"w{nm}", name=f"w{nm}_sb")
                nc.sync.dma_start(out=w_sb, in_=d.ap().rearrange("(t p) o -> p t o", p=128))
                w_sbs[nm] = w_sb

            # ---- GroupNorm, fully per channel-tile (the 16-channel groups
            # never span a 128-channel tile) ----
            eps_sb = sb.tile([8, 1], F32, tag="eps")
            nc.vector.memset(eps_sb, EPS)
            AB = sb.tile([128, NT, 2], F32, tag="AB")
            xn_bf = sb.tile([128, NT, N], BF, tag="xn")
            for t in range(NT):
                stats_t = wkp.tile([128, 2], F32, tag="stats_t", bufs=2, name="stats_t")
                bst = wkp.tile([128, 2, 6], F32, tag="bst")
                for s in range(2):
                    nc.vector.bn_stats(out=bst[:, s, :], in_=x_sb[:, t, s * 512:(s + 1) * 512])
                mv = wkp.tile([128, 2], F32, tag="mv")
                nc.vector.bn_aggr(out=mv, in_=bst)
                # stats_t = [mean, E2]
                nc.vector.tensor_copy(out=stats_t[:, 0:1], in_=mv[:, 0:1])
                nc.vector.tensor_tensor(out=stats_t[:, 1:2], in0=mv[:, 0:1],
                                        in1=mv[:, 0:1], op=mybir.AluOpType.mult)
                nc.vector.tensor_tensor(out=stats_t[:, 1:2], in0=stats_t[:, 1:2],
                                        in1=mv[:, 1:2], op=mybir.AluOpType.add)
                # group [mean, E2] for this tile's 8 groups
                g_ps = ps_pr.tile([8, 2], F32, tag="proj", name="g_ps")
                nc.tensor.matmul(g_ps, sel_sb, stats_t, start=True, stop=True)
                g_sb = wkp.tile([8, 2], F32, tag="g_sb", bufs=2, name="g_sb")
                nc.vector.tensor_copy(out=g_sb, in_=g_ps)
                gvar = wkp.tile([8, 1], F32, tag="gvar", bufs=2, name="gvar")
                nc.vector.tensor_tensor(out=gvar, in0=g_sb[:, 0:1], in1=g_sb[:, 0:1],
                                        op=mybir.AluOpType.mult)
                nc.vector.tensor_tensor(out=gvar, in0=g_sb[:, 1:2], in1=gvar,
                                        op=mybir.AluOpType.subtract)
                nc.scalar.activation(out=gvar, in_=gvar,
                                     func=mybir.ActivationFunctionType.Sqrt,
                                     bias=eps_sb, scale=1.0)
                nc.vector.reciprocal(out=g_sb[:, 1:2], in_=gvar)
                # broadcast [mean, rstd] back to the tile's 128 channels
                bc_ps = ps_pr.tile([128, 2], F32, tag="proj", name="bc_ps")
                nc.tensor.matmul(bc_ps, selT_sb, g_sb, start=True, stop=True)
                # A = rstd * gn_scale ; B = gn_bias - mean * A
                nc.vector.tensor_tensor(out=AB[:, t, 0:1], in0=bc_ps[:, 1:2],
                                        in1=gsc_sb[:, t:t + 1], op=mybir.AluOpType.mult)
                nc.vector.tensor_tensor(out=AB[:, t, 1:2], in0=bc_ps[:, 0:1],
                                        in1=AB[:, t, 0:1], op=mybir.AluOpType.mult)
                nc.vector.tensor_tensor(out=AB[:, t, 1:2], in0=gbi_sb[:, t:t + 1],
                                        in1=AB[:, t, 1:2], op=mybir.AluOpType.subtract)
                for nc5 in range(NC2):
                    sl = slice(nc5 * 512, (nc5 + 1) * 512)
                    eng = nc.gpsimd if t < 2 else nc.vector
                    eng.tensor_scalar(
                        out=xn_bf[:, t, sl], in0=x_sb[:, t, sl],
                        scalar1=AB[:, t, 0:1], scalar2=AB[:, t, 1:2],
                        op0=mybir.AluOpType.mult, op1=mybir.AluOpType.add)

            warm_ps3 = ps_pr.tile([128, 128], F32, tag="proj", name="warm_ps3")
            for _ in range(6):
                nc.tensor.matmul(warm_ps3, xn_bf[:, 0, 0:128],
                                 xn_bf[:, 0, 128:256], start=True, stop=True)

            # ---- projections (helpers) ----
            q_sb = sb.tile([128, NT, N], BF, tag="q")
            k_sb = sb.tile([128, NT, N], BF, tag="k")

            def emit_qk_proj(dst, wname, bias, ot, on_act=False):
                w_sb = w_sbs[wname]
                for nc5 in range(NC2):
                    pp = ps_pr.tile([128, 512], F32, tag="proj", name=f"pr_{wname}")
                    for kc in range(NT):
                        nc.tensor.matmul(
                            pp, w_sb[:, kc, ot * 128:(ot + 1) * 128],
                            xn_bf[:, kc, nc5 * 512:(nc5 + 1) * 512],
                            start=(kc == 0), stop=(kc == NT - 1))
                    out_sl = dst[:, ot, nc5 * 512:(nc5 + 1) * 512]
                    if on_act:
                        nc.scalar.add(out_sl, pp, bias[:, ot:ot + 1])
                    else:
                        nc.vector.tensor_scalar_add(out_sl, pp, bias[:, ot:ot + 1])

            # v^T projection with ones column per head
            vt_sb = sb.tile([128, MT, HEADS, HD + 1], BF, tag="vt")
            nc.vector.memset(vt_sb[:, :, :, HD:HD + 1], 1.0)

            def emit_vt_proj(mt):
                wv_sb = w_sbs["v"]
                pv = ps_pr.tile([128, 512], F32, tag="proj", name="pr_v")
                for kc in range(NT):
                    nc.tensor.matmul(
                        pv, xn_bf[:, kc, mt * 128:(mt + 1) * 128],
                        wv_sb[:, kc, :], start=(kc == 0), stop=(kc == NT - 1))
                nc.vector.tensor_copy(
                    out=vt_sb[:, mt, :, 0:HD],
                    in_=pv.rearrange("p (h c) -> p h c", h=HEADS))

            # ---- attention blocks ----
            xatt_sb = sb.tile([128, NT, N], BF, tag="xatt")
            # denominator staging: both heads at partition 0 (hardware
            # partition_broadcast only reads absolute partition 0), side by
            # side in the free dim
            rs_sb = sb.tile([32, 2, N], F32, tag="rs")
            nc.vector.memset(rs_sb, 1.0)
            rc_sb = sb.tile([32, 2, N], F32, tag="rc")

            pts_all = {}

            def emit_qk_exp(hp, serial_heads=False):
                hA, hB = 2 * hp, 2 * hp + 1
                if serial_heads:
                    order = [(hA, 0, mt) for mt in range(MT)] + \
                            [(hB, 64, mt) for mt in range(MT)]
                else:
                    order = [(h, lo, mt) for mt in range(MT)
                             for h, lo in ((hA, 0), (hB, 64))]
                for h, lo, mt in order:
                    if True:
                        qk = ps_qk.tile([128, N], F32, tag="qk", name="qk_ps")
                        for nc5 in range(NC2):
                            nc.tensor.matmul(
                                qk[:, nc5 * 512:(nc5 + 1) * 512],
                                k_sb[lo:lo + 64, hp, mt * 128:(mt + 1) * 128],
                                q_sb[lo:lo + 64, hp, nc5 * 512:(nc5 + 1) * 512],
                                start=True, stop=True)
                        pt = wkp.tile([128, N], BF, tag="pt", bufs=32, name="pt")
                        nc.scalar.activation(out=pt, in_=qk,
                                             func=mybir.ActivationFunctionType.Exp,
                                             scale=ATT_SCALE)
                        pts_all[(h, mt)] = pt

            def emit_av(hp, head_major=False):
                hA, hB = 2 * hp, 2 * hp + 1
                if head_major:
                    groups = [[(h, nc5) for nc5 in range(NC2)] for h in (hA, hB)]
                else:
                    groups = [[(h, nc5) for h in (hA, hB)] for nc5 in range(NC2)]
                for grp in groups:
                    avs = {}
                    for h, nc5 in grp:
                        sl = slice(nc5 * 512, (nc5 + 1) * 512)
                        av = ps_pr.tile([HD + 1, 512], F32, tag="proj", name="av_ps")
                        for mt in range(MT):
                            nc.tensor.matmul(
                                av, vt_sb[:, mt, h, :],
                                pts_all[(h, mt)][:, sl],
                                start=(mt == 0), stop=(mt == MT - 1))
                        # denominator row (psum row 64) -> partition 0 slot.
                        # staging slot/cols indexed by the group's varying
                        # coordinate (head for chunk-major, chunk for
                        # head-major)
                        if head_major:
                            slot, scol = nc5, slice(0, 512)
                        else:
                            slot, scol = h % 2, sl
                        nc.vector.tensor_copy(out=rs_sb[0:1, slot, scol],
                                              in_=av[HD:HD + 1, :])
                        avs[(h, nc5)] = av

                    # 1/denom for the group's two halves in one op, then
                    # broadcast across 64 partitions on GPSIMD
                    if head_major:
                        nc.vector.reciprocal_approx_fast(
                            out=rc_sb[0:1, 0:2, 0:512],
                            in_=rs_sb[0:1, 0:2, 0:512])
                    else:
                        gsl = slice(grp[0][1] * 512, (grp[0][1] + 1) * 512)
                        nc.vector.reciprocal_approx_fast(out=rc_sb[0:1, :, gsl],
                                                         in_=rs_sb[0:1, :, gsl])
                    for h, nc5 in grp:
                        sl = slice(nc5 * 512, (nc5 + 1) * 512)
                        lo = 64 * (h % 2)
                        if head_major:
                            slot, scol = nc5, slice(0, 512)
                        else:
                            slot, scol = h % 2, sl
                        rb = wkp.tile([64, 512], F32, tag="rb", bufs=8, name="rb")
                        nc.gpsimd.partition_broadcast(
                            out_ap=rb, in_ap=rc_sb[0:1, slot, scol])
                        nc.vector.tensor_tensor(
                            out=xatt_sb[lo:lo + 64, hp, sl],
                            in0=avs[(h, nc5)][0:HD, :], in1=rb,
                            op=mybir.AluOpType.mult)

            # ---- emission order: software pipeline ----
            emit_qk_proj(k_sb, "k", bk_sb, 0, on_act=True)
            emit_qk_proj(q_sb, "q", bq_sb, 0, on_act=True)
            emit_qk_exp(0)
            for mt in range(0, 4):
                emit_vt_proj(mt)
            emit_qk_proj(k_sb, "k", bk_sb, 1)
            emit_qk_proj(q_sb, "q", bq_sb, 1)
            emit_qk_exp(1)
            for mt in range(4, MT):
                emit_vt_proj(mt)
            emit_av(0)
            emit_qk_proj(k_sb, "k", bk_sb, 2)
            emit_qk_proj(q_sb, "q", bq_sb, 2)
            emit_qk_exp(2)
            emit_av(1)
            emit_qk_proj(k_sb, "k", bk_sb, 3)
            emit_qk_proj(q_sb, "q", bq_sb, 3)
            emit_qk_exp(3)
            emit_av(2)

            # final projection pass 1 (kc=0..2) interleaved with the last
            # pair's attn@v so PE never idles during denominator chains;
            # kc=3 joins in pass 2.
            wp_sb = w_sbs["p"]
            y_part = sb.tile([128, NT, N], F32, tag="y_part")

            def emit_pass1(ots):
                for ot in ots:
                    for nc5 in range(NC2):
                        sl = slice(nc5 * 512, (nc5 + 1) * 512)
                        pq = ps_pr.tile([128, 512], F32, tag="proj", name="pr_p1")
                        for kc in range(NT - 1):
                            nc.tensor.matmul(
                                pq, wp_sb[:, kc, ot * 128:(ot + 1) * 128],
                                xatt_sb[:, kc, sl],
                                start=(kc == 0), stop=(kc == NT - 2))
                        # y_part = pq + bias + x
                        nc.vector.scalar_tensor_tensor(
                            out=y_part[:, ot, sl], in0=pq,
                            scalar=bp_sb[:, ot:ot + 1], in1=x_sb[:, ot, sl],
                            op0=mybir.AluOpType.add, op1=mybir.AluOpType.add)

            emit_pass1([0, 1])
            emit_av(HEADS // 2 - 1)
            emit_pass1([2, 3])

            # ---- final projection pass 2 (kc=3) + residual ----
            y_rr = y_d.ap().rearrange("(t p) n -> p t n", p=128)
            for ot in range(NT):
                for nc5 in range(NC2):
                    sl = slice(nc5 * 512, (nc5 + 1) * 512)
                    po = ps_pr.tile([128, 512], F32, tag="proj", name="pr_o")
                    nc.tensor.matmul(
                        po, wp_sb[:, NT - 1, ot * 128:(ot + 1) * 128],
                        xatt_sb[:, NT - 1, sl], start=True, stop=True)
                    yw = wkp.tile([128, 512], F32, tag="yw", bufs=6, name="yw")
                    nc.vector.tensor_tensor(out=yw, in0=po, in1=y_part[:, ot, sl],
                                            op=mybir.AluOpType.add)
                    nc.sync.dma_start(out=y_rr[:, ot, sl], in_=yw)

    nc.compile()
    return nc


_CACHE = {}


def _get_nc(repeat=1):
    key = ("nc", repeat)
    if key not in _CACHE:
        _CACHE[key] = _build_nc(repeat)
    return _CACHE[key]


def _prep_shared(gn_scale, gn_bias, wq, bq, wk, bk, wv, bv, wp, bp):
    def t128(v):
        return np.ascontiguousarray(np.asarray(v, np.float32).reshape(NT, 128).T)

    sel = (np.arange(128)[:, None] // GS == np.arange(8)[None, :]) \
        .astype(np.float32) / GS
    consts = np.concatenate([
        sel, t128(gn_scale), t128(gn_bias), t128(bq), t128(bk),
        t128(np.asarray(wp, np.float64) @ np.asarray(bv, np.float64)
             + np.asarray(bp, np.float64)),
    ], axis=1)
    shared = {
        "wq_t": np.ascontiguousarray(np.asarray(wq, np.float32).T).astype(BF16),
        "wk_t": np.ascontiguousarray(np.asarray(wk, np.float32).T).astype(BF16),
        "wv_t": np.ascontiguousarray(np.asarray(wv, np.float32).T).astype(BF16),
        "wp_t": np.ascontiguousarray(np.asarray(wp, np.float32).T).astype(BF16),
        "consts": np.ascontiguousarray(consts),
        "selT": np.ascontiguousarray(
            (np.arange(8)[:, None] == np.arange(128)[None, :] // GS)
            .astype(np.float32)),
    }
    return shared


def run(inputs, trace=False):
    nc = _get_nc()
    x = np.asarray(inputs["x"], np.float32).reshape(B, C, N)
    shared = _prep_shared(
        inputs["gn_scale"], inputs["gn_bias"], inputs["wq"], inputs["bq"],
        inputs["wk"], inputs["bk"], inputs["wv"], inputs["bv"],
        inputs["wp"], inputs["bp"])
    in_maps = [dict(shared, x=np.ascontiguousarray(x[c])) for c in range(B)]
    try:
        res = run_bass_kernel_spmd(nc, in_maps, core_ids=list(range(B)), trace=trace)
        y = np.stack([res.results[c]["y"] for c in range(B)], axis=0)
    except Exception:
        # transient NRT_EXEC_UNIT_UNRECOVERABLE crashes have been observed on
        # this fabric; a single retry has always succeeded
        res = run_bass_kernel_spmd(nc, in_maps, core_ids=list(range(B)), trace=trace)
        y = np.stack([res.results[c]["y"] for c in range(B)], axis=0)
    return y.reshape(B, C, HH, WW).astype(np.float32), res


def kernel(**inputs) -> np.ndarray:
    y, _ = run(inputs, trace=False)
    return y
